# revision 22
# baseline (speedup 1.0000x reference)
"""Trainium2 Bass kernel for nn_LowRankLinear (y = x @ (U@V).T + bias).

Strategy:
  - Data-parallel: shard the 8192 tokens across 8 NeuronCores (1024 each).
  - Low-rank factorization on-device: t.T = (V @ x.T)  [rank x tok], then
    y.T = U @ t + bias — 34 GFLOP total instead of 283 GFLOP for the
    materialized-W reference.
  - All operands in bf16 (inputs quantized on host, output upcast on host):
    halves all DMA bytes vs fp32 (21 MB/core vs 42 MB) while the PE runs
    bf16 at the same 1 column/cycle as the fp32r path. rel-err ~3.5e-3,
    well within the 2e-2 gate. PSUM accumulation stays fp32.
  - With bf16 the whole x shard (64 KB/partition) is SBUF-resident: no
    tile rotation, no WAR hazards.
  - Critical path is PE busy (256 MMs × ~220 ns ≈ 57 us) + first-data
    latency + drain. So: tiny first transfers (V chunk 0 = 64 KB, x chunk
    0 = 256 KB) dispatched from the scalar/vector sequencers (free at
    ~5.8 us, vs sync's ~7.2 us preamble), a few warm-up matmuls on zeroed
    tiles so the HAM clock-gate reaches full rate before real data lands,
    then a single continuous sync-queue stream sized so DMA stays ahead
    of the PE's ~350 GB/s consumption.
  - y.T orientation makes bias per-PARTITION; PSUM eviction alternates
    DVE (tensor_scalar_add) and ACT (activation Identity+bias).

Self-contained: hardcodes shapes from the problem spec; only needs the
concourse repo at /opt/trn_rl_repo (container-provided).
"""

import sys

if "/opt/trn_rl_repo" not in sys.path:
    sys.path.insert(0, "/opt/trn_rl_repo")

import numpy as np

import concourse.mybir as mybir
import concourse.tile as tile
from concourse import bacc
from concourse.bass_utils import run_bass_kernel_spmd

# Problem shapes (hardcoded per contract)
TOKENS = 8192
IN_F = 4096
OUT_F = 4096
RANK = 256
N_CORES = 8
TPC = TOKENS // N_CORES  # tokens per core = 1024

P = 128  # partitions
NG = 512  # moving free-dim per matmul (PSUM bank limit for f32)
KC = IN_F // P  # 32 k-chunks for matmul1
RC = RANK // P  # 2 rank chunks
G = TPC // NG  # 2 halves of the token range
OFT = OUT_F // P  # 32 out_f tiles for matmul2

# x DMA granularity in HALF-chunks (one half-chunk = [128, 512] = 128 KB):
# tiny first transfers so matmul1 starts ASAP (the serial ~0.7 us HWDGE
# dispatch + ~1.9 us pipe/receipt latency dominates early), then larger
# blocks that stream faster than the PE consumes.
X_HGROUPS = [1, 1, 2, 4, 8, 8, 8, 8, 8, 8, 8]
# V DMA granularity in k-chunks (V chunk = 64 KB)
V_GROUPS = [1, 3, 12, 16]

# HAM warm-up matmuls on zeroed tiles before real data lands: they bridge
# the PE from ~7.5 us (engines ready) to ~13 us (x stream permanently ahead
# of PE consumption), so the HAM clock-gate reaches K=8/8 before the first
# real matmul and never re-throttles (mid-stream PE gaps would reset it).
NWARM = 16

F32 = mybir.dt.float32
MMDT = mybir.dt.bfloat16  # halves all DMA bytes; PE still 1 col/cycle
ODT = mybir.dt.bfloat16  # output stored bf16, upcast to f32 on host

_CACHE = {}


def _build(mmdt):
    nc = bacc.Bacc(
        trn_type="TRN2", target_bir_lowering=False, debug=False, num_devices=N_CORES
    )
    # All inputs pre-packed on host into the exact SBUF images so every DMA
    # is a flat 2D copy with contiguous per-partition lines.
    xP = nc.dram_tensor("xP", [P, KC * TPC], mmdt, kind="ExternalInput")
    vP = nc.dram_tensor("vP", [P, KC * RANK], mmdt, kind="ExternalInput")
    uP = nc.dram_tensor("uP", [P, RC * OUT_F], mmdt, kind="ExternalInput")
    # bias in column layout: bias_col[p, of] = bias[of*128 + p]
    biasc = nc.dram_tensor("biasc", [P, OFT], F32, kind="ExternalInput")
    yT = nc.dram_tensor("yT", [OUT_F, TPC], ODT, kind="ExternalOutput")

    with tile.TileContext(nc) as tc:
        with (
            tc.tile_pool(name="const", bufs=1) as cp,
            tc.tile_pool(name="yp", bufs=6) as yp,
            tc.tile_pool(name="pt", bufs=4, space="PSUM") as ptp,
            tc.tile_pool(name="py", bufs=4, space="PSUM") as pyp,
        ):
            # ---- resident tensors ----
            # Tile deps are per-TILE (not per-range): every tile below is
            # written by exactly ONE dma_start so consumers wait only for
            # the bytes they actually read.
            v_tiles = []  # (cstart, cn, tile): V.T chunk groups
            c0 = 0
            for i, cn in enumerate(V_GROUPS):
                v_tiles.append((c0, cn, cp.tile([P, cn * RANK], mmdt, name=f"vt{i}")))
                c0 += cn
            # U.T quarters: u_tiles[r][half] covers rank-tile r, of half
            u_tiles = [
                [cp.tile([P, OUT_F // 2], mmdt, name=f"ut{r}{h}") for h in range(2)]
                for r in range(RC)
            ]
            # t.T per (r, g): written by one eviction op each
            t_tiles = [
                [cp.tile([P, NG], mmdt, name=f"tt{r}{g}") for g in range(G)]
                for r in range(RC)
            ]
            bcol = cp.tile([P, OFT], F32)  # per-partition bias columns
            wmv = cp.tile([P, NG], mmdt)  # warm-up moving operand (zeros)
            wwt = cp.tile([P, P], mmdt)  # warm-up weights (zeros)
            # whole x shard resident: one tile per DMA group, indexed in
            # half-chunks (half H = 2*c + g covers xP cols [H*NG, (H+1)*NG))
            x_tiles = []
            h0 = 0
            for i, hn in enumerate(X_HGROUPS):
                x_tiles.append((h0, hn, cp.tile([P, hn * NG], mmdt, name=f"xt{i}")))
                h0 += hn

            def v_slice(c):
                # (tile, column offset) for V.T chunk c
                for cstart, cn, vt in v_tiles:
                    if cstart <= c < cstart + cn:
                        return vt, (c - cstart) * RANK
                raise AssertionError(c)

            def x_slice(c, g):
                # (tile, column offset) for x.T half-chunk (c, g)
                h = 2 * c + g
                for hstart, hn, xt in x_tiles:
                    if hstart <= h < hstart + hn:
                        return xt, (h - hstart) * NG
                raise AssertionError((c, g))

            # ---- PE warm-up: zeroed tiles, no DMA dependency ----
            nc.vector.memset(wmv[:], 0.0)
            nc.vector.memset(wwt[:], 0.0)
            wpt = pyp.tile([P, NG], F32, name="warm", tag="py")
            for _ in range(NWARM):
                nc.tensor.matmul(wpt[:], wwt[:], wmv[:], start=True, stop=True)

            def load_v(eng, i):
                cstart, cn, vt = v_tiles[i]
                eng.dma_start(
                    vt[:], vP[:, cstart * RANK : (cstart + cn) * RANK]
                )

            def load_u(eng, r, half):
                sl = slice(
                    r * OUT_F + half * (OUT_F // 2),
                    r * OUT_F + (half + 1) * (OUT_F // 2),
                )
                eng.dma_start(u_tiles[r][half][:], uP[:, sl])

            def load_x(eng, i):
                hstart, hn, xt = x_tiles[i]
                eng.dma_start(xt[:], xP[:, hstart * NG : (hstart + hn) * NG])

            # ---- inflow ----
            # ALL bulk transfers go on the sync ring in FIFO priority order:
            # the SDMA engines starve the Act ring when the sync ring has fat
            # packets queued, so a second ring only helps for the tiny bias.
            # V groups are interleaved just ahead of the x chunks needing them.
            load_x(nc.sync, 0)  # x c0 g0 (128 KB) — first matmul gate
            load_v(nc.sync, 0)  # V c0 (64 KB)
            load_x(nc.sync, 1)  # x c0 g1
            load_x(nc.sync, 2)  # x c1
            load_v(nc.sync, 1)  # V c1-3
            nc.scalar.dma_start(bcol[:], biasc[:])  # tiny, needed at ~39 us
            load_x(nc.sync, 3)  # x c2-3
            load_v(nc.sync, 2)  # V c4-15
            load_x(nc.sync, 4)  # x c4-7
            load_x(nc.sync, 5)  # x c8-11
            load_v(nc.sync, 3)  # V c16-31
            load_x(nc.sync, 6)  # x c12-15
            load_x(nc.sync, 7)  # x c16-19
            load_x(nc.sync, 8)  # x c20-23
            load_u(nc.sync, 0, 0)  # U for of 0-15 ...
            load_u(nc.sync, 1, 0)  # ... lands well before needed (~40 us)
            load_x(nc.sync, 9)  # x c24-27
            load_x(nc.sync, 10)  # x c28-31
            load_u(nc.sync, 0, 1)  # U for of 16-31
            load_u(nc.sync, 1, 1)

            # ---- matmul1: t.T = sum_c V.T_c.T @ x.T_c over both token halves ----
            pt = [
                ptp.tile([P, NG], F32, name=f"pt{r}_{g}", tag="pt")
                for r in range(RC)
                for g in range(G)
            ]
            for c in range(KC):
                if c == 0 or c == KC - 1:
                    # first chunk: g0 pair first (x c0 g1 arrives later);
                    # last chunk: g-major so g0 PSUM tiles stop first
                    order = [(r, g) for g in range(G) for r in range(RC)]
                else:
                    order = [(r, g) for r in range(RC) for g in range(G)]
                vt, voff = v_slice(c)
                for r, g in order:
                    xt, xoff = x_slice(c, g)
                    nc.tensor.matmul(
                        pt[r * G + g][:],
                        vt[:, voff + r * P : voff + (r + 1) * P],
                        xt[:, xoff : xoff + NG],
                        start=(c == 0),
                        stop=(c == KC - 1),
                    )
            # f32 PSUM -> bf16 SBUF rounding copies; g-major so matmul2's g0
            # operands are ready first; r0 on DVE, r1 on ACT in parallel.
            for g in range(G):
                for r in range(RC):
                    if r == 0:
                        nc.vector.tensor_copy(t_tiles[r][g][:], pt[r * G + g][:])
                    else:
                        nc.scalar.copy(t_tiles[r][g][:], pt[r * G + g][:])

            # ---- matmul2: y.T[of] = U.T_of.T @ t.T + bias ----
            # Eviction alternates DVE / ACT so both engines share the load.
            for of in range(OFT):
                last = of == OFT - 1
                if last:
                    # last of-tile: independent half tiles, each stored as
                    # soon as it's evicted, so the final DMA is only 128 KB
                    # (a shared tile would serialize store(g0) before the
                    # g1 eviction through a per-tile WAR dependency)
                    yhalf = [yp.tile([P, NG], ODT, name=f"ylast{g}") for g in range(G)]
                else:
                    ysb = yp.tile([P, TPC], ODT)
                for g in range(G):
                    pyt = pyp.tile([P, NG], F32, tag="py")
                    for r in range(RC):
                        half = of // (OFT // 2)
                        lof = of - half * (OFT // 2)
                        nc.tensor.matmul(
                            pyt[:],
                            u_tiles[r][half][:, lof * P : (lof + 1) * P],
                            t_tiles[r][g][:],
                            start=(r == 0),
                            stop=(r == RC - 1),
                        )
                    dst = yhalf[g][:] if last else ysb[:, g * NG : (g + 1) * NG]
                    if g == 0:
                        nc.vector.tensor_scalar_add(dst, pyt[:], bcol[:, of : of + 1])
                    else:
                        nc.scalar.activation(
                            dst,
                            pyt[:],
                            mybir.ActivationFunctionType.Identity,
                            bias=bcol[:, of : of + 1],
                        )
                    if last:
                        nc.sync.dma_start(
                            yT[of * P : (of + 1) * P, g * NG : (g + 1) * NG],
                            yhalf[g][:],
                        )
                if not last:
                    nc.sync.dma_start(yT[of * P : (of + 1) * P, :], ysb[:])
    nc.compile()
    return nc


def _get_nc():
    key = MMDT
    if key not in _CACHE:
        _CACHE[key] = _build(key)
    return _CACHE[key]


def _prep_in_maps(x, U, V, bias):
    import ml_dtypes

    bf16 = ml_dtypes.bfloat16
    # Cast to bf16 first so the pack-transposes move half the bytes.
    x = np.asarray(x, dtype=np.float32).astype(bf16)
    V = np.asarray(V, dtype=np.float32).astype(bf16)
    U = np.asarray(U, dtype=np.float32).astype(bf16)
    # SBUF images: vsb[p, c*RANK+m] = V[m, c*128+p]; usb[p, r*OUT_F+o] = U[o, r*128+p]
    vp = np.ascontiguousarray(
        V.reshape(RANK, KC, P).transpose(2, 1, 0).reshape(P, KC * RANK)
    )
    up = np.ascontiguousarray(
        U.reshape(OUT_F, RC, P).transpose(2, 1, 0).reshape(P, RC * OUT_F)
    )
    bc = np.ascontiguousarray(np.asarray(bias, dtype=np.float32).reshape(OFT, P).T)
    in_maps = []
    for i in range(N_CORES):
        xs = x[i * TPC : (i + 1) * TPC, :]
        # xP[p, c*TPC+n] = x[n, c*128+p]
        xp_img = np.ascontiguousarray(
            xs.reshape(TPC, KC, P).transpose(2, 1, 0).reshape(P, KC * TPC)
        )
        in_maps.append({"xP": xp_img, "vP": vp, "uP": up, "biasc": bc})
    return in_maps


def _gather(res):
    # res.results[i]["yT"] is [OUT_F, TPC] bf16; full y is the token-major
    # concat of the transposes, upcast to f32 on host.
    yt = np.concatenate([res.results[i]["yT"] for i in range(N_CORES)], axis=1)
    return np.ascontiguousarray(yt.astype(np.float32).T)


def kernel(x, U, V, bias):
    nc = _get_nc()
    in_maps = _prep_in_maps(x, U, V, bias)
    res = run_bass_kernel_spmd(nc, in_maps, core_ids=list(range(N_CORES)))
    return _gather(res)


def run_profiled(x, U, V, bias, **trace_kwargs):
    """Like kernel() but with NTFF tracing; returns (y, BassKernelResults)."""
    nc = _get_nc()
    in_maps = _prep_in_maps(x, U, V, bias)
    res = run_bass_kernel_spmd(
        nc, in_maps, core_ids=list(range(N_CORES)), trace=True, **trace_kwargs
    )
    return _gather(res), res


# revision 27
# speedup vs baseline: 1.0178x; 1.0178x over previous
"""Trainium2 Bass kernel for nn_LowRankLinear (y = x @ (U@V).T + bias).

Strategy:
  - Data-parallel: shard the 8192 tokens across 8 NeuronCores (1024 each).
  - Low-rank factorization on-device: t.T = (V @ x.T)  [rank x tok], then
    y.T = U @ t + bias — 34 GFLOP total instead of 283 GFLOP for the
    materialized-W reference.
  - All operands in bf16 (inputs quantized on host, output upcast on host):
    halves all DMA bytes vs fp32 (21 MB/core vs 42 MB) while the PE runs
    bf16 at the same 1 column/cycle as the fp32r path. rel-err ~3.5e-3,
    well within the 2e-2 gate. PSUM accumulation stays fp32.
  - Batch-pipelined phases over two 512-token halves (b0 = tokens 0-511,
    b1 = 512-1023), with x packed g-major on the host so b0's bytes stream
    first:
      A: mm1(b0) rides the V+x(b0) inflow (DMA-bound, PE ~83% duty)
      B: mm2(b0) on resident t/U while x(b1)+U stream behind
      C: mm1(b1) on resident x — zero DMA dependence
      D: mm2(b1) + store drain
    This overlaps mm2 work with the unavoidable inflow time instead of
    serializing all 256 matmuls behind it, and keeps PE gaps <100 ns so
    the HAM clock-gate never re-throttles to half rate.
  - Tile deps are per-TILE: every tile is written by exactly one dma_start.
  - All bulk DMA on the sync ring in FIFO priority order (the SDMA engines
    starve the Act ring when the sync ring is loaded).
  - y.T orientation makes bias per-PARTITION; PSUM eviction alternates
    DVE (tensor_scalar_add) and ACT (activation Identity+bias) per of.
    Stores are batched 2 of-tiles per DMA; a deep y pool buffers the
    phase-B store backlog while inflow saturates the HBM bandwidth.

Self-contained: hardcodes shapes from the problem spec; only needs the
concourse repo at /opt/trn_rl_repo (container-provided).
"""

import sys

if "/opt/trn_rl_repo" not in sys.path:
    sys.path.insert(0, "/opt/trn_rl_repo")

import numpy as np

import concourse.mybir as mybir
import concourse.tile as tile
from concourse import bacc
from concourse.bass_utils import run_bass_kernel_spmd

# Problem shapes (hardcoded per contract)
TOKENS = 8192
IN_F = 4096
OUT_F = 4096
RANK = 256
N_CORES = 8
TPC = TOKENS // N_CORES  # tokens per core = 1024

P = 128  # partitions
NG = 512  # moving free-dim per matmul (PSUM bank limit for f32)
KC = IN_F // P  # 32 k-chunks for matmul1
RC = RANK // P  # 2 rank chunks
G = TPC // NG  # 2 token batches
OFT = OUT_F // P  # 32 out_f tiles for matmul2

# x DMA granularity per batch, in half-chunks (one = [128, 512] = 128 KB).
XB_GROUPS = [
    [1, 1, 2, 4, 8, 8, 8],  # b0: tiny first transfers for a fast launch
    [8, 8, 8, 8],  # b1: coarse, arrives during phase B
]
# V DMA granularity in k-chunks (V chunk = 64 KB)
V_GROUPS = [1, 3, 12, 16]

NWARM = 6  # HAM warm-up matmuls on zeroed tiles before real data lands

F32 = mybir.dt.float32
MMDT = mybir.dt.bfloat16  # halves all DMA bytes; PE still 1 col/cycle
ODT = mybir.dt.bfloat16  # output stored bf16, upcast to f32 on host

_CACHE = {}


def _build(mmdt):
    nc = bacc.Bacc(
        trn_type="TRN2", target_bir_lowering=False, debug=False, num_devices=N_CORES
    )
    # All inputs pre-packed on host into the exact SBUF images so every DMA
    # is a flat 2D copy with contiguous per-partition lines.
    # xP is g-major: xP[p, (g*KC + c)*NG + n] = x[g*NG + n, c*128 + p]
    xP = nc.dram_tensor("xP", [P, KC * TPC], mmdt, kind="ExternalInput")
    vP = nc.dram_tensor("vP", [P, KC * RANK], mmdt, kind="ExternalInput")
    uP = nc.dram_tensor("uP", [P, RC * OUT_F], mmdt, kind="ExternalInput")
    # bias in column layout: bias_col[p, of] = bias[of*128 + p]
    biasc = nc.dram_tensor("biasc", [P, OFT], F32, kind="ExternalInput")
    # partition-major output: yTp[p, ((b*16 + op)*2 + q)*NG + n] =
    # y[b*NG + n, (2*op + q)*128 + p] — every store is a flat 2D copy
    yTp = nc.dram_tensor("yTp", [P, OFT * TPC], ODT, kind="ExternalOutput")

    with tile.TileContext(nc) as tc:
        with (
            tc.tile_pool(name="const", bufs=1) as cp,
            tc.tile_pool(name="yp", bufs=18) as yp,
            tc.tile_pool(name="pt", bufs=2, space="PSUM") as ptp,
            tc.tile_pool(name="py", bufs=5, space="PSUM") as pyp,
        ):
            # ---- resident tensors (one DMA writer per tile) ----
            v_tiles = []  # (cstart, cn, tile): V.T chunk groups
            c0 = 0
            for i, cn in enumerate(V_GROUPS):
                v_tiles.append((c0, cn, cp.tile([P, cn * RANK], mmdt, name=f"vt{i}")))
                c0 += cn
            # U.T quarters: u_tiles[r][half] covers rank-tile r, of half
            u_tiles = [
                [cp.tile([P, OUT_F // 2], mmdt, name=f"ut{r}{h}") for h in range(2)]
                for r in range(RC)
            ]
            # t.T per (r, b): written by one eviction op each
            t_tiles = [
                [cp.tile([P, NG], mmdt, name=f"tt{r}{b}") for b in range(G)]
                for r in range(RC)
            ]
            bcol = cp.tile([P, OFT], F32)  # per-partition bias columns
            wmv = cp.tile([P, NG], mmdt)  # warm-up moving operand (zeros)
            wwt = cp.tile([P, P], mmdt)  # warm-up weights (zeros)
            # whole x shard resident: one tile per DMA group, per batch
            x_tiles = [[], []]  # [b] -> list of (hstart, hn, tile)
            for b in range(G):
                h0 = 0
                for i, hn in enumerate(XB_GROUPS[b]):
                    x_tiles[b].append(
                        (h0, hn, cp.tile([P, hn * NG], mmdt, name=f"x{b}_{i}"))
                    )
                    h0 += hn

            def v_slice(c):
                for cstart, cn, vt in v_tiles:
                    if cstart <= c < cstart + cn:
                        return vt, (c - cstart) * RANK
                raise AssertionError(c)

            def x_slice(b, c):
                for hstart, hn, xt in x_tiles[b]:
                    if hstart <= c < hstart + hn:
                        return xt, (c - hstart) * NG
                raise AssertionError((b, c))

            # ---- PE warm-up: zeroed tiles, no DMA dependency ----
            nc.vector.memset(wmv[:], 0.0)
            nc.vector.memset(wwt[:], 0.0)
            wpt = pyp.tile([P, NG], F32, name="warm", tag="py")
            for _ in range(NWARM):
                nc.tensor.matmul(wpt[:], wwt[:], wmv[:], start=True, stop=True)

            def load_v(i):
                cstart, cn, vt = v_tiles[i]
                nc.sync.dma_start(vt[:], vP[:, cstart * RANK : (cstart + cn) * RANK])

            def load_u(r, half):
                sl = slice(
                    r * OUT_F + half * (OUT_F // 2),
                    r * OUT_F + (half + 1) * (OUT_F // 2),
                )
                nc.sync.dma_start(u_tiles[r][half][:], uP[:, sl])

            def load_x(b, i):
                hstart, hn, xt = x_tiles[b][i]
                base = b * KC * NG
                nc.sync.dma_start(
                    xt[:], xP[:, base + hstart * NG : base + (hstart + hn) * NG]
                )

            # ---- inflow (sync ring, FIFO priority order) ----
            load_x(0, 0)  # x b0 c0 (128 KB) — first matmul gate
            load_v(0)  # V c0 (64 KB)
            load_x(0, 1)  # x b0 c1
            load_x(0, 2)  # x b0 c2-3
            load_v(1)  # V c1-3
            nc.scalar.dma_start(bcol[:], biasc[:])  # tiny, Act ring
            load_x(0, 3)  # x b0 c4-7
            load_v(2)  # V c4-15
            load_x(0, 4)  # x b0 c8-15
            load_v(3)  # V c16-31
            load_x(0, 5)  # x b0 c16-23
            load_x(0, 6)  # x b0 c24-31
            load_u(0, 0)  # U needed from phase B start (~27 us)
            load_u(1, 0)
            load_u(0, 1)
            load_u(1, 1)
            load_x(1, 0)  # x b1, consumed from phase C (~41 us)
            load_x(1, 1)
            load_x(1, 2)
            load_x(1, 3)

            pt = [None, None]

            def mm1(b):
                # t.T[r, b] = sum_c V.T_c.T @ x.T_c(b)
                pt[b] = [
                    ptp.tile([P, NG], F32, name=f"pt{r}_{b}", tag="pt")
                    for r in range(RC)
                ]
                for c in range(KC):
                    vt, voff = v_slice(c)
                    xt, xoff = x_slice(b, c)
                    for r in range(RC):
                        nc.tensor.matmul(
                            pt[b][r][:],
                            vt[:, voff + r * P : voff + (r + 1) * P],
                            xt[:, xoff : xoff + NG],
                            start=(c == 0),
                            stop=(c == KC - 1),
                        )
                # f32 PSUM -> bf16 SBUF rounding copies, DVE + ACT in parallel
                nc.vector.tensor_copy(t_tiles[0][b][:], pt[b][0][:])
                nc.scalar.copy(t_tiles[1][b][:], pt[b][1][:])

            def mm2(b):
                # y.T[of, b] = U.T_of.T @ t.T(b) + bias; 2 of-tiles per store
                for op in range(OFT // 2):
                    ysb = yp.tile([P, 2 * NG], ODT, tag="ys")
                    for q in range(2):
                        of = 2 * op + q
                        half = of // (OFT // 2)
                        lof = of - half * (OFT // 2)
                        pyt = pyp.tile([P, NG], F32, tag="py")
                        for r in range(RC):
                            nc.tensor.matmul(
                                pyt[:],
                                u_tiles[r][half][:, lof * P : (lof + 1) * P],
                                t_tiles[r][b][:],
                                start=(r == 0),
                                stop=(r == RC - 1),
                            )
                        dst = ysb[:, q * NG : (q + 1) * NG]
                        if q == 0:
                            nc.vector.tensor_scalar_add(
                                dst, pyt[:], bcol[:, of : of + 1]
                            )
                        else:
                            nc.scalar.activation(
                                dst,
                                pyt[:],
                                mybir.ActivationFunctionType.Identity,
                                bias=bcol[:, of : of + 1],
                            )
                    slot = b * (OFT // 2) + op
                    nc.sync.dma_start(
                        yTp[:, slot * 2 * NG : (slot + 1) * 2 * NG], ysb[:]
                    )

            mm1(0)  # phase A: rides the V+x(b0) inflow
            mm2(0)  # phase B: overlaps x(b1)+U inflow
            mm1(1)  # phase C: resident data, no DMA dependence
            mm2(1)  # phase D: store drain
    nc.compile()
    return nc


def _get_nc():
    key = MMDT
    if key not in _CACHE:
        _CACHE[key] = _build(key)
    return _CACHE[key]


def _prep_in_maps(x, U, V, bias):
    import ml_dtypes

    bf16 = ml_dtypes.bfloat16
    # Cast to bf16 first so the pack-transposes move half the bytes.
    x = np.asarray(x, dtype=np.float32).astype(bf16)
    V = np.asarray(V, dtype=np.float32).astype(bf16)
    U = np.asarray(U, dtype=np.float32).astype(bf16)
    # SBUF images: vsb[p, c*RANK+m] = V[m, c*128+p]; usb[p, r*OUT_F+o] = U[o, r*128+p]
    vp = np.ascontiguousarray(
        V.reshape(RANK, KC, P).transpose(2, 1, 0).reshape(P, KC * RANK)
    )
    up = np.ascontiguousarray(
        U.reshape(OUT_F, RC, P).transpose(2, 1, 0).reshape(P, RC * OUT_F)
    )
    bc = np.ascontiguousarray(np.asarray(bias, dtype=np.float32).reshape(OFT, P).T)
    in_maps = []
    for i in range(N_CORES):
        xs = x[i * TPC : (i + 1) * TPC, :]
        # g-major: xP[p, (g*KC + c)*NG + n] = xs[g*NG + n, c*128 + p]
        xp_img = np.ascontiguousarray(
            xs.reshape(G, NG, KC, P).transpose(3, 0, 2, 1).reshape(P, KC * TPC)
        )
        in_maps.append({"xP": xp_img, "vP": vp, "uP": up, "biasc": bc})
    return in_maps


def _gather(res):
    # res.results[i]["yTp"] is [P, 32768] bf16 in partition-major slot
    # layout; decode per core to [TPC, OUT_F] and concat over tokens.
    parts = []
    for i in range(N_CORES):
        arr = res.results[i]["yTp"]
        yc = (
            arr.reshape(P, G, OFT // 2, 2, NG)
            .transpose(1, 4, 2, 3, 0)
            .reshape(TPC, OUT_F)
        )
        parts.append(yc)
    return np.concatenate(parts, axis=0).astype(np.float32)


def kernel(x, U, V, bias):
    nc = _get_nc()
    in_maps = _prep_in_maps(x, U, V, bias)
    res = run_bass_kernel_spmd(nc, in_maps, core_ids=list(range(N_CORES)))
    return _gather(res)


def run_profiled(x, U, V, bias, **trace_kwargs):
    """Like kernel() but with NTFF tracing; returns (y, BassKernelResults)."""
    nc = _get_nc()
    in_maps = _prep_in_maps(x, U, V, bias)
    res = run_bass_kernel_spmd(
        nc, in_maps, core_ids=list(range(N_CORES)), trace=True, **trace_kwargs
    )
    return _gather(res), res
